# revision 141
# baseline (speedup 1.0000x reference)
"""Trainium2 Bass kernel for nn_BinaryMLP_90881507983459.

Binarized MLP: 4 layers, all matmuls contract sign(+-1) values.
  h1 = sign(x) @ sign(w1).T + b1 ; t1 = sign-of-BN(h1)   (eps=1e-5)
  h2 = t1 @ sign(w2).T + b2      ; t2 = sign-of-BN(h2)   (eps=1e-5)
  h3 = t2 @ sign(w3).T + b3      ; t3 = sign-of-BN(h3)   (eps=512.0)
  out = t3 @ sign(w4).T + b4
Key facts used:
  * hardtanh never matters: only sign() of each BN output feeds the next
    matmul, and the BN scale g*rsqrt(v+eps) is strictly positive, so
    sign(BN(h)) == (h >= th) with th = m - be/s - b folded per channel.
  * +-1 products accumulate exactly in PSUM fp32, so fp8 sign inputs are
    lossless; x is cast-loaded as fp8e5 and binarized via its SIGN BIT
    (IEEE rounding never flips the sign bit): out = (u16 & 0x8080) | 0x3838
    maps each fp8 byte pair to +-1.0 in fp8e4 exactly, and runs at DVE
    4x (2-byte packed, all-SBUF) or on Pool.
  * L1's threshold th1 is folded INTO the matmul: the 240 zero-padded
    contraction rows of w1 (784 -> 1024) get a 6-term exact fp8
    decomposition of -(ceil(th1)-0.5) against constant +1.0 moving rows
    (xs pad pairs 392..394 memset to 0x3838), so PSUM = h1 - th1* and
    the L1 bnsign is a PURE sign (immediate bias, no threshold load).
  * Activation transposes [b,f] -> [f,b] go through the DMA xbar
    (InstDmaTransposeAnt, 16x128 u16 tiles) for steady-state groups; the
    first two groups (pipeline fill, DMA engines saturated by x/w loads)
    transpose on the then-idle PE instead (bf16-typed u16 pairs via
    identity matmul, PSUM->SBUF copy on ACT/DVE).
Layout: transposed activations [C(partitions), B(free)]; batch sharded
8 ways (2048 rows/core); weights/thresholds preprocessed on host and
replicated. Uneven groups (256,512,512,512,256) shorten pipeline fill
and drain; iter i runs L1(i), L2(i-1), L3(i-2), L4(i-3) so every
bnsign gets ~a full iteration of slack. L4's bias is folded via a
rank-1 ones @ b4 product; the identity for the fill-phase PE
transposes ships inside the packed constants.
"""

import sys

sys.path.insert(0, "/opt/trn_rl_repo")

import numpy as np
import ml_dtypes

import concourse.bass as bass  # noqa: F401  (bass must import before bacc)
import concourse.bacc as bacc
import concourse.tile as tile
import concourse.mybir as mybir
import concourse.masks as masks
from concourse.bass_utils import run_bass_kernel_spmd

BF16 = ml_dtypes.bfloat16
FP8NP = mybir.dt.np(mybir.dt.float8e4)
F32 = mybir.dt.float32
BF = mybir.dt.bfloat16
U16 = mybir.dt.uint16
FP8 = mybir.dt.float8e4
FP8E5 = mybir.dt.float8e5
U8 = mybir.dt.uint8
ALU = mybir.AluOpType
DR = mybir.MatmulPerfMode.DoubleRow

B, D_IN, H, D_OUT = 16384, 784, 512, 10
NCORES = 8
BL = B // NCORES  # 2048 rows per core
KIN1 = 4          # 1024 = 4*(128*2) padded feature pair-subtiles for L1 fp8 DR
KH = 4            # 512 = 4*128 hidden subtiles
NPAIR = D_IN // 2  # 392 real u16 feature pairs per row
GROUPS = [(0, 256), (256, 512), (768, 512), (1280, 512), (1792, 256)]
W_FP8 = 4096 + 2048 + 2048 + KH * D_OUT   # packed fp8 weights per partition
W_F32 = 8 + D_OUT + 64                    # thresholds + bias + bf16 identity

# bnsign per m on every layer: even m on ACT (+-1), odd m on DVE is_ge
# (+-0.5, the consumer's k-subtiles doubled on host).
HALF_SET = {1: (1, 3), 2: (1, 3), 3: (1, 3)}


def _body(nc, tc, pools, aps):
    xfp, xsp, xTp, actp, psp, ps4p, wp = pools
    x_ap, out_ap, wkp, tkp = aps

    ng = len(GROUPS)
    xs_tiles = {}
    xT_tiles = {}

    # --- constants / staging buffers -----------------------------------
    # per-partition scalar masks for the bitwise binarize (ptr operands so
    # the ALU runs in u16, not an f32 immediate)
    msk_and = wp.tile([128, 1], U16, tag="mska", name="mska")
    msk_or = wp.tile([128, 1], U16, tag="msko", name="msko")
    nc.vector.memset(msk_and[:], 0x8080)
    nc.vector.memset(msk_or[:], 0x3838)
    ones = wp.tile([1, 128], F32, tag="ones", name="ones")
    nc.vector.memset(ones[:], 1.0)

    # per-group xs staging buffers: [b-part, j, 512 u16 feature pairs];
    # pairs 392..394 = +1.0 fp8 pairs (the constant rows that multiply the
    # th1 decomposition in w1's padding), 395..511 zero pad. Pads for
    # g0-g2 on DVE (done before xf0 lands), g3/g4 on Pool.
    xs_bufs = [xsp.tile([128, 4, 512], U16, tag=f"xs{i}", name=f"xsb{i}")
               for i in range(len(GROUPS))]
    for xsb in xs_bufs[:3]:
        nc.vector.memset(xsb[:, :, NPAIR:NPAIR + 3], 0x3838)
        nc.vector.memset(xsb[:, :, NPAIR + 3:], 0)

    def issue_load(g):
        # one SWDGE cast-load per group: f32 HBM -> fp8e5 SBUF (cost model
        # counts the fp8 side). Sign bit survives the cast exactly.
        b0, w = GROUPS[g]
        nj = w // 128
        xf = xfp.tile([128, 4, D_IN], FP8E5, tag="xf", name=f"xf{g}")[:, :nj, :]
        xin = x_ap[b0:b0 + w, :].rearrange("(j p) f -> p j f", p=128)
        nc.gpsimd.dma_start(xf, xin)
        return xf

    xf_tiles = {}

    def issue_bin(g, eng, islt=False):
        # binarize into xs. DVE path: bitwise (in&0x8080)|0x3838 -> +-1.0
        # fp8 at 4x rate. Pool path (no bitwise ISA): u8 is_lt sign-bit
        # test -> +-0.5 fp8; those groups' fold columns hold +0.5 so the
        # whole L1 PSUM is scaled by 0.5, which the pure sign ignores.
        b0, w = GROUPS[g]
        nj = w // 128
        xs = xs_bufs[g][:, :nj, :]
        if islt:
            xfu8 = xf_tiles[g].bitcast(U8)
            eng.tensor_scalar(xs.bitcast(FP8)[:, :, :D_IN], xfu8, 128, 0.5,
                              ALU.is_lt, ALU.subtract)
        else:
            xfu = xf_tiles[g].bitcast(U16)
            eng.tensor_scalar(xs[:, :, :NPAIR], xfu, msk_and[:], msk_or[:],
                              ALU.bitwise_and, ALU.bitwise_or)
        xs_tiles[g] = xs

    def issue_transposes_dma(g):
        # ONE DMA xbar transpose for the whole group: xs is contiguous, so
        # [128 rows, nj*512 u16] -> [128, nj*4 chunks, 128 rows]; chunk
        # c = j*4+kq holds feature pairs [128*kq, 128*kq+128) of rows
        # j*128..j*128+127. One HWDGE slot + one completion sem per group.
        b0, w = GROUPS[g]
        nj = w // 128
        xT = xTp.tile([128, 16, 128], U16, tag="xT16", name=f"xT{g}")
        nc.sync.dma_start_transpose(xT[:, :4 * nj, :], xs_tiles[g])
        xT_tiles[g] = ("T16", xT, nj)

    def issue_transposes_pe(g):
        # fill-phase path: PE is_transpose of bf16-typed u16 pairs into a
        # single-slot PSUM staging bank, then PSUM->SBUF copies on ACT/DVE
        # (all idle during fill; the DMA engines are busy with x/w loads).
        b0, w = GROUPS[g]
        nj = w // 128
        xT = xTp.tile([128, 4, 512], U16, tag="xT", name=f"xT{g}")
        xsb = xs_tiles[g].bitcast(BF)
        xTc = xT.bitcast(F32)  # [128, 4, 256]
        for kh in range(2):
            xb = ps4p.tile([128, 2, 512], BF, tag="psb", name=f"xb{g}_{kh}")
            for kq in (2 * kh, 2 * kh + 1):
                for j in range(nj):
                    nc.tensor.transpose(xb[:, kq - 2 * kh, j * 128:(j + 1) * 128],
                                        xsb[:, j, kq * 128:(kq + 1) * 128], idn)
            if kh == 0 or g <= 1:
                nc.scalar.copy(xTc[:, 2 * kh:2 * kh + 2, :w // 2],
                               xb.bitcast(F32)[:, :, :w // 2])
            else:
                nc.vector.tensor_copy(xTc[:, 2:4, :w // 2], xb.bitcast(F32)[:, :, :w // 2])
        xT_tiles[g] = ("classic", xT, nj)

    # --- packed weights ------------------------------------------------
    wk = wp.tile([128, W_FP8], FP8, tag="wk", name="wk")
    tk = wp.tile([128, W_F32], F32, tag="tk", name="tk")
    w1s = wk[:, 0:4096].rearrange("p (k r o) -> p k r o", k=KIN1, r=2)
    w2s = wk[:, 4096:6144].rearrange("p (k o) -> p k o", k=KH)
    w3s = wk[:, 6144:8192].rearrange("p (k o) -> p k o", k=KH)
    w4s = wk[:, 8192:8192 + KH * D_OUT].rearrange("p (k o) -> p k o", k=KH)
    th2s = tk[:, 0:4]
    th3s = tk[:, 4:8]
    b4s = tk[0:1, 8:18]
    idn = tk[:, 18:].bitcast(BF)  # [128, 128] identity, host-packed

    # --- prologue. Two hard constraints shape the issue order:
    # (1) DMA-transposes are fenced behind ALL earlier-issued passthrough
    #     DMAs (xbar mode switch);
    # (2) the serial DMA engine is FIFO-by-readiness, so weight chunks
    #     are slotted between the early x transfers via their queues.
    xf_tiles[0] = issue_load(0)
    xf_tiles[1] = issue_load(1)
    nc.scalar.dma_start(tk[:], tkp)
    nc.scalar.dma_start(wk[:, 0:2048], wkp[:, 0:2048])        # w1 m01
    nc.scalar.dma_start(wk[:, 2048:4096], wkp[:, 2048:4096])  # w1 m23
    nc.scalar.dma_start(wk[:, 4096:6144], wkp[:, 4096:6144])  # w2
    xf_tiles[2] = issue_load(2)
    issue_bin(0, nc.vector)
    xf_tiles[3] = issue_load(3)
    issue_bin(1, nc.vector)
    # pads for the late groups on Pool, AFTER its descgen burst (program
    # position keeps the scheduler from running them before xf0's desc).
    for xsb in xs_bufs[3:]:
        nc.gpsimd.memset(xsb[:, :, NPAIR:NPAIR + 3], 0x3838)
        nc.gpsimd.memset(xsb[:, :, NPAIR + 3:], 0)

    # group 0 transposes on the idle PE
    issue_transposes_pe(0)

    a1 = [[None] * ng for _ in range(2)]
    a2 = [[None] * ng for _ in range(2)]
    a3 = [[None] * ng for _ in range(2)]

    def bnsign(dst, src, m, th):
        # per-m BN-sign; even m on ACT (+-1), odd m on DVE is_ge (+-0.5,
        # consumer k-subtiles doubled on host). th=None -> pure sign (L1).
        if m % 2 == 0:
            nc.scalar.sign(dst, src, bias=(0.0 if th is None else th[:, m:m + 1]))
        else:
            nc.vector.tensor_scalar(dst, src, (0.0 if th is None else th[:, m:m + 1]),
                                    0.5, ALU.is_ge, ALU.subtract)

    def layer1(g):
        # fp8 DoubleRow; th1 pre-folded into w1 pad rows -> pure sign
        b0, w = GROUPS[g]
        kind, xT, nj = xT_tiles[g]
        if kind == "classic":
            xTf = xT.bitcast(FP8).rearrange("p k (b r) -> p k r b", r=2)
            movs = [xTf[:, kq, :, :w] for kq in range(KIN1)]
        else:
            xTf = xT[:, :4 * nj, :].bitcast(FP8).rearrange(
                "p (j k) (b r) -> p k r j b", k=KIN1, r=2)
            movs = [xTf[:, kq] for kq in range(KIN1)]
        for mp in range(2):
            a1[mp][g] = actp.tile([128, 2, w], FP8, tag=f"a1_{mp}_{g}",
                                  name=f"a1_{mp}_{g}")
        for m in range(KH):
            ps = psp.tile([128, 512], F32, tag="acc", name=f"ps1_{g}_{m}")[:, :w]
            for kq in range(KIN1):
                nc.tensor.matmul(ps, w1s[:, kq, :, m * 128:(m + 1) * 128],
                                 movs[kq],
                                 start=(kq == 0), stop=(kq == KIN1 - 1),
                                 perf_mode=DR)
            bnsign(a1[m // 2][g][:, m % 2, :], ps, m, None)

    def layer23(g, ws, ths, src_a, dst_a, tagp):
        # fp8 DoubleRow, kp-major over the pair-tiles of the layer above
        b0, w = GROUPS[g]
        for mp in range(2):
            dst_a[mp][g] = actp.tile([128, 2, w], FP8, tag=f"{tagp}_{mp}_{g}",
                                     name=f"{tagp}_{mp}_{g}")
        pss = [psp.tile([128, 512], F32, tag="acc", name=f"ps_{tagp}_{g}_{m}")[:, :w]
               for m in range(KH)]
        for kp in range(2):
            for m in range(KH):
                nc.tensor.matmul(pss[m],
                                 ws[:, 2 * kp:2 * kp + 2, m * 128:(m + 1) * 128],
                                 src_a[kp][g][:], start=(kp == 0), stop=(kp == 1),
                                 perf_mode=DR)
        for m in range(KH):
            bnsign(dst_a[m // 2][g][:, m % 2, :], pss[m], m, ths)

    def layer4(g):
        # fp8 normal mode; bias folded in via ones @ b4
        b0, w = GROUPS[g]
        nj = w // 128
        ps4 = ps4p.tile([128, 512], F32, tag="psb", name=f"ps4_{g}")
        ps4v = ps4[:, :nj * D_OUT].rearrange("p (j o) -> p j o", o=D_OUT)
        for j in range(nj):
            nc.tensor.matmul(ps4v[:, j, :], ones[:], b4s[:], start=True, stop=False)
            for k in range(KH):
                nc.tensor.matmul(ps4v[:, j, :],
                                 a3[k // 2][g][:, k % 2, j * 128:(j + 1) * 128],
                                 w4s[:, k, :], start=False, stop=(k == KH - 1))
        ob = actp.tile([128, 4, D_OUT], F32, tag=f"ob{g}", name=f"ob{g}")
        if g >= ng - 2:  # epilogue: ACT is busy with L3 bnsigns, DVE idles
            nc.vector.tensor_copy(ob[:, :nj, :], ps4v[:, :nj, :])
        else:
            nc.scalar.copy(ob[:, :nj, :], ps4v[:, :nj, :])
        nc.sync.dma_start(out_ap[:, b0 // 128:b0 // 128 + nj, :], ob[:, :nj, :])

    # --- deep software pipeline: iter i runs L1(i), L2(i-1), L3(i-2),
    # L4(i-3) so each layer's bnsigns get most of an iteration of slack
    # and rarely stall the in-order PE queue. -----------------------------
    for i in range(ng + 3):
        if i + 4 < ng:
            xf_tiles[i + 4] = issue_load(i + 4)
        if i == 0:
            nc.scalar.dma_start(wk[:, 6144:8192], wkp[:, 6144:8192])  # w3
        if i == 1:
            nc.scalar.dma_start(wk[:, 8192:], wkp[:, 8192:])          # w4
        if i < ng:
            layer1(i)
        # transposes for i+1 AFTER L1(i)'s matmuls: the PE-path transposes
        # wait on bin(i+1) and would otherwise stall the in-order PE queue
        if i + 1 < ng:
            if i + 1 < 2:
                issue_transposes_pe(i + 1)
            else:
                issue_transposes_dma(i + 1)
        if i >= ng and 0 <= i - 3:
            layer4(i - 3)
        if 0 <= i - 1 < ng:
            layer23(i - 1, w2s, th2s, a1, a2, "a2")
        if 0 <= i - 2 < ng:
            layer23(i - 2, w3s, th3s, a2, a3, "a3")
        if i < ng and 0 <= i - 3:
            layer4(i - 3)
        if 2 <= i + 2 < ng:
            # bins at END of iter: bin2 on DVE behind this iter's
            # bnsigns; bins 3,4 on Pool
            issue_bin(i + 2, nc.vector)


def build(repeat=1):
    nc = bacc.Bacc("TRN2", target_bir_lowering=False, debug=False)
    x = nc.dram_tensor("x", [BL, D_IN], F32, kind="ExternalInput")
    wkp = nc.dram_tensor("wkp", [128, W_FP8], FP8, kind="ExternalInput")
    tkp = nc.dram_tensor("tkp", [128, W_F32], F32, kind="ExternalInput")
    out = nc.dram_tensor("out", [BL, D_OUT], F32, kind="ExternalOutput")

    out_ap = out.ap().rearrange("(j p) o -> p j o", p=128)

    with tile.TileContext(nc) as tc:
        with tc.tile_pool(name="w", bufs=1) as wp, \
             tc.tile_pool(name="xf", bufs=5) as xfp, \
             tc.tile_pool(name="xs", bufs=1) as xsp, \
             tc.tile_pool(name="xT", bufs=3) as xTp, \
             tc.tile_pool(name="act", bufs=1) as actp, \
             tc.tile_pool(name="ps", bufs=7, space="PSUM") as psp, \
             tc.tile_pool(name="ps4", bufs=1, space="PSUM") as ps4p:
            pools = (xfp, xsp, xTp, actp, psp, ps4p, wp)
            for _ in range(repeat):
                _body(nc, tc, pools, (x.ap(), out_ap, wkp.ap(), tkp.ap()))
    nc.compile()
    return nc


def _sgn(a):
    return np.where(a >= 0, np.float32(1), np.float32(-1))


def _decompose_th(th):
    """-(ceil(th)-0.5) as the sum of 6 exact fp8e4m3 values [6, H].

    This fp8e4m3 is the IEEE variant (max 240): four multiple-of-16
    terms clipped to +-240 cover +-960, then an integer and a +-0.5."""
    t = -(np.ceil(th) - 0.5)
    t = np.clip(t, -788.5, 788.5)
    r = t
    terms = []
    for _ in range(4):
        v = np.clip(np.round(r / 16.0) * 16.0, -240, 240)
        terms.append(v)
        r = r - v
    v5 = np.round(r)
    terms.append(v5)
    terms.append(r - v5)
    terms = np.stack(terms)
    assert np.all(terms.sum(axis=0) == t), "th decomposition not exact"
    assert np.all(np.abs(r) <= 8.5)
    back = terms.astype(FP8NP).astype(np.float64)
    assert np.all(back == terms), "th terms not fp8-exact"
    return terms


def prep_weights(w1, b1, g1, be1, m1, v1, w2, b2, g2, be2, m2, v2,
                 w3, b3, g3, be3, m3, v3, w4, b4):
    """Host-side constant preprocessing (weights only, no x-dependent work)."""
    def wpack(w, ksub):  # w [O, I] -> [128, ksub, O] sign, zero-padded
        O, I = w.shape
        arr = np.zeros((ksub * 128, O), dtype=FP8NP)
        arr[:I] = _sgn(w).T.astype(FP8NP)
        return np.ascontiguousarray(arr.reshape(ksub, 128, O).transpose(1, 0, 2))

    def thraw(b, g, be, m, v, eps):
        # sign(BN(h)) == h >= th,  th = m - be/(g*rsqrt(v+eps)) - b
        s = g.astype(np.float64) / np.sqrt(v.astype(np.float64) + eps)
        return m.astype(np.float64) - be.astype(np.float64) / s - b.astype(np.float64)

    def thpack(th, half_set):
        # column m: ACT sign wants bias=-th; DVE is_ge wants +th
        out = np.ascontiguousarray((-th).astype(np.float32).reshape(KH, 128).T)
        for m in half_set:
            out[:, m] *= -1.0
        return out

    def double_k(wpk, half_set):
        # +-0.5-emitting k-subtiles of the producing layer: double the rows
        out = wpk.astype(np.float32)
        for k in half_set:
            out[:, k, :] *= 2.0
        return out.astype(FP8NP)

    O, I = w1.shape
    arr = np.zeros((KIN1 * 256, O), dtype=FP8NP)  # feature f = 2*(kq*128+p)+r
    arr[:I] = _sgn(w1).T.astype(FP8NP)
    # th1 fold: rows 784..789 (against constant +1.0 moving features
    # 784..789)
    th1 = thraw(b1, g1, be1, m1, v1, 1e-5)
    arr[D_IN:D_IN + 6] = _decompose_th(th1).astype(FP8NP)
    w1pk = np.ascontiguousarray(
        arr.reshape(KIN1, 128, 2, O).transpose(1, 0, 2, 3))
    wkp = np.concatenate([
        w1pk.reshape(128, -1),
        double_k(wpack(w2, KH), HALF_SET[1]).reshape(128, -1),
        double_k(wpack(w3, KH), HALF_SET[2]).reshape(128, -1),
        double_k(wpack(w4, KH), HALF_SET[3]).reshape(128, -1),
    ], axis=1)
    idn = np.eye(128, dtype=BF16).view(np.float32)  # [128, 64] f32 view
    tkp = np.concatenate([
        thpack(thraw(b2, g2, be2, m2, v2, 1e-5), HALF_SET[2]),
        thpack(thraw(b3, g3, be3, m3, v3, 512.0), HALF_SET[3]),
        np.broadcast_to(b4.astype(np.float32), (128, D_OUT)),
        idn,
    ], axis=1)
    return {
        "wkp": np.ascontiguousarray(wkp),
        "tkp": np.ascontiguousarray(tkp.astype(np.float32)),
    }


_nc_cache = {}


def get_nc(repeat=1):
    if repeat not in _nc_cache:
        _nc_cache[repeat] = build(repeat)
    return _nc_cache[repeat]


def kernel(x, w1, b1, g1, be1, m1, v1, w2, b2, g2, be2, m2, v2,
           w3, b3, g3, be3, m3, v3, w4, b4):
    nc = get_nc(1)
    consts = prep_weights(w1, b1, g1, be1, m1, v1, w2, b2, g2, be2, m2, v2,
                          w3, b3, g3, be3, m3, v3, w4, b4)
    x = np.ascontiguousarray(np.asarray(x, dtype=np.float32))
    in_maps = [dict(consts, x=x[c * BL:(c + 1) * BL]) for c in range(NCORES)]
    res = run_bass_kernel_spmd(nc, in_maps, core_ids=list(range(NCORES)))
    return np.concatenate([res.results[c]["out"] for c in range(NCORES)], axis=0)
